# revision 12
# baseline (speedup 1.0000x reference)
"""Trainium2 Bass kernel for batched multi-head attention (B=2, S=2048, E=1024, H=16).

Sharding: core r = 4*b + g handles batch b and head-group g (4 heads, 256 emb cols).
- QKV projections: tensor-parallel over head groups (each core computes its 256
  output cols from the full 1024-dim input). K then Q (transposed [e,t] layout,
  bias fused into the ACT-engine PSUM evacuation), then V (token-major, mask
  scale fused at evacuation). Input loads are spread across three DMA queues
  (sync/vector/scalar) so the descriptor feed is not serialized.
- Attention: 4 heads as two pairs (mt). Scores kept transposed [kt, qt]; the
  two heads of a pair run as row-tiled (64-contraction) matmuls emitted
  adjacently so they share the PE array. Softmax exp is split across engines:
  head 0 of each pair on the ACT engine (two 512-wide chunks to cut the
  dependence latency), head 1 on the DVE via a Schraudolph fp16 exp
  (y = bitcast_fp16(int16(x*A + B)), rel err ~1.8% rms, final output err
  ~5e-3). Normalization is deferred via a mask/ones column appended to V;
  reciprocal on the DVE (no ACT table swap), broadcast on GpSimd.
- Out-proj: token-parallel. Contexts exchanged with a per-pair 4-rank
  AllToAll inside each batch's core group (half the wire bytes of an 8-rank
  exchange and no cross-batch zero padding); receivers DMA the slots straight
  into SBUF. Pair-0 out-proj matmuls overlap the final collective.
The whole datapath runs in fp16 (full-rate PE streaming + fast weight load,
half DMA bytes); all matmul accumulation stays in fp32 PSUM.
"""

import sys

if '/opt/trn_rl_repo' not in sys.path:
    sys.path.insert(0, '/opt/trn_rl_repo')

import numpy as np

P = 128
B, S, E, H, DH = 2, 2048, 1024, 16, 64
NCORES = 8
G = 4                 # head groups == cores per batch
EG = E // G           # 256 emb cols per group
TS = S // G           # 512 tokens per core in out-proj
KB = S // P           # 16 key-token blocks
IB = E // P           # 8 contraction blocks of 128
QW = 512              # matmul moving free-dim chunk
HW = 1024             # query half width in attention
SCALE = DH ** -0.5
# Schraudolph fp16 exp: i16 = x*EXPA + EXPB; bitcast to fp16 ~= exp(x*SCALE).
# 1024*log2(e)*SCALE; offset 15360 - 58 (C=58 centers the sawtooth error),
# +0.5 so truncating converts round correctly.
EXPA = float(1024.0 * np.log2(np.e) * SCALE)
EXPB = 15360.0 - 58.0 + 0.5

_cache = {}


def _build():
    import concourse.bass as bass
    import concourse.mybir as mybir
    import concourse.tile as tile
    from concourse import bacc
    from contextlib import ExitStack

    f32 = mybir.dt.float32
    f16 = mybir.dt.float16
    i16 = mybir.dt.int16
    AF = mybir.ActivationFunctionType
    ALU = mybir.AluOpType

    nc = bacc.Bacc("TRN2", target_bir_lowering=False, debug=False,
                   num_devices=NCORES)

    xqT = nc.dram_tensor("xqT", [E, S], f16, kind="ExternalInput").ap()
    xkT = nc.dram_tensor("xkT", [E, S], f16, kind="ExternalInput").ap()
    xvT = nc.dram_tensor("xvT", [E, S], f16, kind="ExternalInput").ap()
    wqT = nc.dram_tensor("wqT", [E, EG], f16, kind="ExternalInput").ap()
    wkT = nc.dram_tensor("wkT", [E, EG], f16, kind="ExternalInput").ap()
    wvT = nc.dram_tensor("wvT", [E, EG], f16, kind="ExternalInput").ap()
    woT = nc.dram_tensor("woT", [E, E], f16, kind="ExternalInput").ap()
    # packed f16 constants: [ones(512) | bv(256) | bo(1024)]
    cpack = nc.dram_tensor("cpack", [1, QW + EG + E], f16,
                           kind="ExternalInput").ap()
    # packed f32 per-partition constants:
    # [mask_pb(16) | maskrep(64) | bq_m0 | bq_m1 | bk_m0 | bk_m1 | zmask(8)]
    MQ0 = KB + KB * G
    cpackP = nc.dram_tensor("cpackP", [P, MQ0 + 4 + NCORES], f32,
                            kind="ExternalInput").ap()
    out = nc.dram_tensor("out", [TS, E], f32, kind="ExternalOutput").ap()

    a2a_ins = [nc.dram_tensor(f"a2a_in{mt}", [NCORES, P, TS], f16).ap()
               for mt in range(2)]
    a2a_outs = [nc.dram_tensor(f"a2a_out{mt}", [NCORES, P, TS], f16).ap()
                for mt in range(2)]
    groups = [list(range(NCORES))]

    with tile.TileContext(nc) as tc, ExitStack() as top:
        const = top.enter_context(tc.tile_pool(name="const", bufs=1))

        cpk = const.tile([1, QW + EG + E], f16)
        ones_b = cpk[:, 0:QW]
        bv_r = cpk[:, QW:QW + EG]
        bo_r = cpk[:, QW + EG:QW + EG + E]
        mpk = const.tile([P, MQ0 + 4 + NCORES], f32)
        mask_t = mpk[:, 0:KB]
        maskrep_t = mpk[:, KB:KB + KB * G]
        bias_q = [mpk[:, MQ0 + m:MQ0 + m + 1] for m in range(2)]
        bias_k = [mpk[:, MQ0 + 2 + m:MQ0 + 3 + m] for m in range(2)]
        zmask_t = mpk[:, MQ0 + 4:MQ0 + 4 + NCORES]

        # persistent projection outputs
        proj_sb = top.enter_context(tc.tile_pool(name="proj_sb", bufs=1))
        qpT = [proj_sb.tile([P, S], f16, tag=f"qpT{m}", name=f"qpT{m}")
               for m in range(2)]
        kpT = [proj_sb.tile([P, S], f16, tag=f"kpT{m}", name=f"kpT{m}")
               for m in range(2)]
        # vp tiles: per kt-block, [P, 4 heads x (64 vals + 1 mask col)]
        vp_sb = [proj_sb.tile([P, G * (DH + 1)], f16, tag=f"vp{m}", name=f"vp{m}")
                 for m in range(KB)]

        # out-proj weights on the gpsimd queue (idle early)
        wo_pool = top.enter_context(tc.tile_pool(name="wo", bufs=1))
        wo_r = wo_pool.tile([P, IB * E], f16)

        with tc.tile_pool(name="wqkv", bufs=1) as wqkv, \
             tc.tile_pool(name="xk", bufs=1) as xkp, \
             tc.tile_pool(name="xq", bufs=1) as xqp, \
             tc.tile_pool(name="xv", bufs=1) as xvp:
            nc.gpsimd.dma_start(cpk[:], cpack[:])
            nc.gpsimd.dma_start(mpk[:], cpackP[:])
            # weight tiles [P, IB*EG]: per i-block, 2 m-halves of 128 cols
            wk_r = wqkv.tile([P, IB * EG], f16, tag="wkr", name="wkr")
            wq_r = wqkv.tile([P, IB * EG], f16, tag="wqr", name="wqr")
            wv_r = wqkv.tile([P, IB * EG], f16, tag="wvr", name="wvr")
            xk = [xkp.tile([P, S], f16, tag=f"xk{i}", name=f"xk{i}")
                  for i in range(IB)]
            xq = [xqp.tile([P, S], f16, tag=f"xq{i}", name=f"xq{i}")
                  for i in range(IB)]
            xv = [xvp.tile([P, S], f16, tag=f"xv{i}", name=f"xv{i}")
                  for i in range(IB)]
            # interleave weight-slice + x-block loads; three queues in parallel
            for i in range(IB):
                isl = slice(i * P, (i + 1) * P)
                esl = slice(i * EG, (i + 1) * EG)
                nc.sync.dma_start(wk_r[:, esl], wkT[isl, :])
                for c in range(2):
                    csl = slice(c * HW, (c + 1) * HW)
                    nc.sync.dma_start(xk[i][:, csl], xkT[isl, csl])
                nc.gpsimd.dma_start(wq_r[:, esl], wqT[isl, :])
                for c in range(2):
                    csl = slice(c * HW, (c + 1) * HW)
                    nc.gpsimd.dma_start(xq[i][:, csl], xqT[isl, csl])
                nc.scalar.dma_start(wv_r[:, esl], wvT[isl, :])
                for c in range(2):
                    csl = slice(c * HW, (c + 1) * HW)
                    nc.scalar.dma_start(xv[i][:, csl], xvT[isl, csl])
            for i in range(IB):
                for c in range(2):
                    nc.gpsimd.dma_start(
                        wo_r[c * 64:(c + 1) * 64, i * E:(i + 1) * E],
                        woT[i * P + c * 64:i * P + (c + 1) * 64, :])

            # ---- K and Q projections: out [e_sel, t] transposed ----
            # bias is added by the ACT engine during PSUM evacuation.
            with tc.tile_pool(name="kqpsum", bufs=1, space="PSUM") as kqpsum:
                for name, xr, wr, biases, dsts in (
                        ("k", xk, wk_r, bias_k, kpT),
                        ("q", xq, wq_r, bias_q, qpT)):
                    kqs = [kqpsum.tile([P, S], f32, tag=f"kqs{m}", name=f"kqs{m}")
                           for m in range(2)]
                    for i in range(IB):
                        for m in range(2):
                            for c in range(S // QW):
                                nc.tensor.matmul(
                                    kqs[m][:, c * QW:(c + 1) * QW],
                                    wr[:, i * EG + m * P:i * EG + (m + 1) * P],
                                    xr[i][:, c * QW:(c + 1) * QW],
                                    start=(i == 0), stop=(i == IB - 1))
                    for m in range(2):
                        nc.scalar.activation(dsts[m][:], kqs[m][:],
                                             AF.Identity, bias=biases[m])

            # ---- V projection, token-major: out [t, e] directly ----
            with tc.tile_pool(name="vpsum", bufs=4, space="PSUM") as vpsum:
                for t in range(KB):
                    vps = vpsum.tile([P, EG], f32)
                    for i in range(IB):
                        nc.tensor.matmul(
                            vps[:], xv[i][:, t * P:(t + 1) * P],
                            wv_r[:, i * EG:(i + 1) * EG],
                            start=(i == 0), stop=False)
                    nc.tensor.matmul(
                        vps[:], ones_b[:, 0:P], bv_r[:],
                        start=False, stop=True)
                    dst3 = vp_sb[t].rearrange("p (h e) -> p h e", e=DH + 1)
                    nc.vector.tensor_scalar_mul(
                        dst3[:, :, 0:DH],
                        vps.rearrange("p (h e) -> p h e", e=DH),
                        mask_t[:, t:t + 1])
                    nc.vector.tensor_copy(
                        dst3[:, :, DH:DH + 1],
                        maskrep_t[:, t * G:(t + 1) * G]
                        .rearrange("p (h e) -> p h e", e=1))

        # ga[ib] holds emb rows [ib*128, (ib+1)*128) of the concat context =
        # head pair ib%2 of group ib//2; DMA'd straight from the a2a output.
        gap = top.enter_context(tc.tile_pool(name="gap", bufs=1))
        ga = {}

        # ---- attention: head pair (2mt, 2mt+1) ----
        # PSUM budget: sp0/sp1 [128,1024] (2 banks each) + pv0/pv1 [65,1024]
        # (2 banks each) = 8 banks, all at bufs=1.
        with tc.tile_pool(name="spsum", bufs=1, space="PSUM") as spsum, \
             tc.tile_pool(name="pvpsum", bufs=1, space="PSUM") as pvpsum, \
             tc.tile_pool(name="expp", bufs=1) as expp, \
             tc.tile_pool(name="normp", bufs=2) as normp, \
             tc.tile_pool(name="ctxp", bufs=2) as ctxp, \
             tc.tile_pool(name="sendp", bufs=8) as sendp:
            for mt in range(2):
                for half in range(2):
                    q0 = half * HW
                    pv = [pvpsum.tile([DH + 1, HW], f32, tag=f"pv{hh}",
                                      name=f"pv{hh}") for hh in range(2)]

                    def emit_scores(j, slot):
                        sps = [spsum.tile([P, HW], f32, tag=f"sp{hh}",
                                          name=f"sp{hh}") for hh in range(2)]
                        # chunk-paired emission: same-c matmuls of the two
                        # heads are adjacent, so their 64-row tiles overlap
                        # in the PE array.
                        for c in range(HW // QW):
                            for hh in range(2):
                                po = hh * DH
                                nc.tensor.matmul(
                                    sps[hh][:, c * QW:(c + 1) * QW],
                                    kpT[mt][po:po + DH, j * P:(j + 1) * P],
                                    qpT[mt][po:po + DH,
                                            q0 + c * QW:q0 + (c + 1) * QW],
                                    start=True, stop=True)
                        # head 0: ACT exp in 2 chunks (latency); head 1:
                        # DVE Schraudolph into an int16 view of the f16 tile.
                        es0 = expp.tile([P, HW], f16, tag=f"es0_{slot}",
                                        name=f"es0_{slot}")
                        for c in range(HW // QW):
                            csl = slice(c * QW, (c + 1) * QW)
                            nc.scalar.activation(es0[:, csl], sps[0][:, csl],
                                                 AF.Exp, scale=SCALE)
                        es1 = expp.tile([P, HW], f16, tag=f"es1_{slot}",
                                        name=f"es1_{slot}")
                        nc.vector.tensor_scalar(
                            out=es1[:].bitcast(i16), in0=sps[1][:],
                            scalar1=EXPA, scalar2=EXPB,
                            op0=ALU.mult, op1=ALU.add)
                        return [es0, es1]

                    es_cur = emit_scores(0, 0)
                    for j in range(KB):
                        if j + 1 < KB:
                            es_nxt = emit_scores(j + 1, (j + 1) % 2)
                        for hh in range(2):
                            h = 2 * mt + hh
                            for c in range(HW // QW):
                                nc.tensor.matmul(
                                    pv[hh][:, c * QW:(c + 1) * QW],
                                    vp_sb[j][:, h * (DH + 1):(h + 1) * (DH + 1)],
                                    es_cur[hh][:, c * QW:(c + 1) * QW],
                                    start=(j == 0), stop=(j == KB - 1))
                        es_cur = es_nxt

                    # normalize: DVE reciprocal (no ACT table swap), GpSimd
                    # broadcast in 512-chunks, DVE muls into the send tile
                    # (both heads stacked -> each slot is a single DMA).
                    rec = [normp.tile([1, HW], f32, tag=f"rec{hh}",
                                      name=f"rec{hh}") for hh in range(2)]
                    recB = [normp.tile([DH, HW], f32, tag=f"recB{hh}",
                                       name=f"recB{hh}") for hh in range(2)]
                    ctx2 = ctxp.tile([P, HW], f16)
                    for hh in range(2):
                        nc.vector.reciprocal(rec[hh][:], pv[hh][DH:DH + 1, :])
                        for c in range(2):
                            csl = slice(c * QW, (c + 1) * QW)
                            nc.gpsimd.partition_broadcast(recB[hh][:, csl],
                                                          rec[hh][:, csl])
                    for c in range(2):
                        csl = slice(c * QW, (c + 1) * QW)
                        for hh in range(2):
                            nc.vector.tensor_mul(
                                ctx2[hh * DH:(hh + 1) * DH, csl],
                                pv[hh][0:DH, csl], recB[hh][:, csl])
                    # slots: my-batch pair (s, s+4); zmask zeroes the
                    # cross-batch copy so the receiver's pair-sum works.
                    for jj in range(2):
                        shard = half * 2 + jj
                        for slot in (shard, shard + 4):
                            st = sendp.tile([P, TS], f16)
                            nc.vector.tensor_scalar_mul(
                                st[:], ctx2[:, jj * TS:(jj + 1) * TS],
                                zmask_t[:, slot:slot + 1])
                            nc.sync.dma_start(a2a_ins[mt][slot], st[:])
                # one exchange for the whole pair
                nc.gpsimd.collective_compute(
                    "AllToAll", mybir.AluOpType.bypass,
                    replica_groups=groups,
                    ins=[a2a_ins[mt][:]], outs=[a2a_outs[mt][:]])
                if mt == 0:
                    # pair-0 slots land in SBUF while pair-1 attention runs;
                    # my-batch slot + zeroed cross-batch slot are pair-summed.
                    for gp in range(G):
                        ib = gp * 2
                        gt = gap.tile([P, TS], f16, tag=f"ga{ib}",
                                      name=f"ga{ib}")
                        t0 = sendp.tile([P, TS], f16)
                        t1 = sendp.tile([P, TS], f16)
                        nc.sync.dma_start(t0[:], a2a_outs[0][gp])
                        nc.sync.dma_start(t1[:], a2a_outs[0][gp + 4])
                        nc.vector.tensor_add(gt[:], t0[:], t1[:])
                        ga[ib] = gt

        # ---- out-proj on my 512-token slice ----
        # phase 0 (pair-0 rows, gathered long ago) overlaps the final
        # AllToAll; phase 1 consumes pair-1 right after its gather.
        with tc.tile_pool(name="opsum", bufs=1, space="PSUM") as opsum, \
             tc.tile_pool(name="outsb", bufs=2) as outsb:
            pot = [opsum.tile([P, E], f32, tag=f"pot{tm}", name=f"pot{tm}")
                   for tm in range(TS // P)]
            for tm in range(TS // P):
                for n, ib in enumerate((0, 2, 4, 6)):
                    for oc in range(E // QW):
                        nc.tensor.matmul(
                            pot[tm][:, oc * QW:(oc + 1) * QW],
                            ga[ib][:, tm * P:(tm + 1) * P],
                            wo_r[:, ib * E + oc * QW:ib * E + oc * QW + QW],
                            start=(n == 0), stop=False)
            for gp in range(G):
                ib = gp * 2 + 1
                gt = gap.tile([P, TS], f16, tag=f"ga{ib}", name=f"ga{ib}")
                t0 = outsb.tile([P, TS], f16, tag="g0", name="g0")
                t1 = outsb.tile([P, TS], f16, tag="g1", name="g1")
                nc.sync.dma_start(t0[:], a2a_outs[1][gp])
                nc.sync.dma_start(t1[:], a2a_outs[1][gp + 4])
                nc.vector.tensor_add(gt[:], t0[:], t1[:])
                ga[ib] = gt
            for tm in range(TS // P):
                for ib in (1, 3, 5, 7):
                    for oc in range(E // QW):
                        nc.tensor.matmul(
                            pot[tm][:, oc * QW:(oc + 1) * QW],
                            ga[ib][:, tm * P:(tm + 1) * P],
                            wo_r[:, ib * E + oc * QW:ib * E + oc * QW + QW],
                            start=False, stop=False)
                for oc in range(E // QW):
                    nc.tensor.matmul(
                        pot[tm][:, oc * QW:(oc + 1) * QW],
                        ones_b[:, 0:P],
                        bo_r[:, oc * QW:(oc + 1) * QW],
                        start=False, stop=True)
                ot = outsb.tile([P, E], f32)
                nc.scalar.activation(ot[:], pot[tm][:], AF.Copy)
                for c in range(4):
                    eng = nc.sync if c % 2 == 0 else nc.scalar
                    eng.dma_start(
                        out[tm * P + c * 32:tm * P + (c + 1) * 32, :],
                        ot[c * 32:(c + 1) * 32, :])

    nc.compile()
    return nc


def _get_nc():
    if 'nc' not in _cache:
        _cache['nc'] = _build()
    return _cache['nc']


def kernel(q, k, v, mask, Wq, bq, Wk, bk, Wv, bv, Wo, bo):
    from concourse.bass_utils import run_bass_kernel_spmd

    nc = _get_nc()
    f32 = np.float32
    f16 = np.float16
    q = np.asarray(q, f32)
    k = np.asarray(k, f32)
    v = np.asarray(v, f32)

    qT = [np.ascontiguousarray(q[b].T).astype(f16) for b in range(B)]
    kT = [np.ascontiguousarray(k[b].T).astype(f16) for b in range(B)]
    vT = [np.ascontiguousarray(v[b].T).astype(f16) for b in range(B)]
    WqT = np.asarray(Wq, f32).T.astype(f16)
    WkT = np.asarray(Wk, f32).T.astype(f16)
    WvT = np.asarray(Wv, f32).T.astype(f16)
    WoT = np.asarray(Wo, f32).T.astype(f16)
    bq = np.asarray(bq, f32)
    bk = np.asarray(bk, f32)
    bv = np.asarray(bv, f32).astype(f16)
    bo = np.asarray(bo, f32).astype(f16)
    onesv = np.ones((QW,), f16)
    maskf = (np.asarray(mask) != 0).astype(f32)  # [B, S]

    in_maps = []
    for r in range(NCORES):
        b, g = r // G, r % G
        cols = slice(g * EG, (g + 1) * EG)
        m_pb = np.ascontiguousarray(maskf[b].reshape(KB, P).T)       # [128,16]
        m_rep = np.ascontiguousarray(np.repeat(m_pb, G, axis=1))     # [128,64]
        bq_c = bq[cols].reshape(2, P).T                              # [128,2]
        bk_c = bk[cols].reshape(2, P).T                              # [128,2]
        zm = np.zeros((P, NCORES), f32)
        zm[:, b * G:(b + 1) * G] = 1.0
        cpk = np.concatenate([onesv, bv[cols], bo],
                             axis=0)[None, :].astype(f16)
        cpkP = np.concatenate([m_pb, m_rep, bq_c, bk_c, zm],
                              axis=1).astype(f32)
        in_maps.append({
            "xqT": qT[b], "xkT": kT[b], "xvT": vT[b],
            "wqT": np.ascontiguousarray(WqT[:, cols]),
            "wkT": np.ascontiguousarray(WkT[:, cols]),
            "wvT": np.ascontiguousarray(WvT[:, cols]),
            "woT": WoT,
            "cpack": np.ascontiguousarray(cpk),
            "cpackP": np.ascontiguousarray(cpkP),
        })

    res = run_bass_kernel_spmd(nc, in_maps, core_ids=list(range(NCORES)),
                               **_cache.get('run_kwargs', {}))
    _cache['last_results'] = res

    full = np.empty((B, S, E), f32)
    for r in range(NCORES):
        b, g = r // G, r % G
        full[b, g * TS:(g + 1) * TS, :] = res.results[r]["out"]
    return full
